# revision 2
# baseline (speedup 1.0000x reference)
"""Trainium2 Bass kernel for ColRepeatCausalLinear:

    decay   = clip(decay_value, 0.9, 1.0)
    cache_t = decay * cache_{t-1} + x_t          (scan along T, per (b, d))
    out_t   = weight[t] * cache_t + bias[t]

Shapes: x [B=8, T=4096, D=1024] f32, weight [1, T], bias [T], decay_value [1].

Strategy (one batch per NeuronCore, 8 cores):
  - Chunk T into 32 blocks of 128.  Within a chunk the scan is a matmul
    with the lower-triangular decay matrix L[t, s] = decay^(t-s) (t >= s):
    cache_k = L @ x_k, computed on the TensorEngine in fp32 with the
    chunk rows on partitions and D on the free axis (no transposes).
  - The cross-chunk carry folds into the next chunk's matmul through
    row 0: since L[t, 0] = decay^t, adding decay*carry to x_{k+1}[0, :]
    makes L @ x' produce the full prefix (carry term decay^{t+1}*carry).
    carry = cache_k[127, :].  Engine APs must start on a 32-aligned
    partition, so the matmul's M ordering is rotated by one (PSUM
    partition 0 holds cache[127], partition m holds cache[m-1]); the
    carry patch then reads PSUM partition 0 and the output DMA
    un-rotates (rows 1..127 in one transfer, row 0 in a small second).
  - D is split into two halves (one PSUM bank each) forming two
    independent carry chains, interleaved so the PE never idles on the
    serial patch latency.
  - Final out = weight[t]*cache + bias[t] is a single ScalarEngine
    activation (Identity) with per-partition scale/bias APs, PSUM -> SBUF.
"""

import numpy as np

B, T, D = 8, 4096, 1024
CH = 128                 # chunk rows (PE contraction dim)
NK = T // CH             # 32 chunks
CPG = 4                  # chunks per DMA staging group
NG = NK // CPG           # 8 staging groups
NH = 2                   # d-halves (carry chains)
DH = D // NH             # 512 = one PSUM bank of fp32

_CACHED = {}


def _build_program(decay: float):
    import concourse.bass as bass
    import concourse.mybir as mybir
    from concourse import bacc
    from concourse.tile import TileContext

    f32 = mybir.dt.float32
    nc = bacc.Bacc("TRN2", target_bir_lowering=False)

    x_d = nc.dram_tensor("x", [T, D], f32, kind="ExternalInput")
    lt_d = nc.dram_tensor("lt", [CH, CH], f32, kind="ExternalInput")
    w_d = nc.dram_tensor("w", [CH, NK], f32, kind="ExternalInput")
    b_d = nc.dram_tensor("b", [CH, NK], f32, kind="ExternalInput")
    y_d = nc.dram_tensor("y", [T, D], f32, kind="ExternalOutput")

    with TileContext(nc) as tc:
        with (
            tc.tile_pool(name="const", bufs=1) as const,
            tc.tile_pool(name="xin", bufs=3) as xpool,
            tc.tile_pool(name="oout", bufs=3) as opool,
            tc.tile_pool(name="psum", bufs=2, space="PSUM") as pspool,
        ):
            lt = const.tile([CH, CH], f32)
            nc.sync.dma_start(out=lt[:], in_=lt_d[:])
            wsb = const.tile([CH, NK], f32)
            nc.sync.dma_start(out=wsb[:], in_=w_d[:])
            bsb = const.tile([CH, NK], f32)
            nc.sync.dma_start(out=bsb[:], in_=b_d[:])

            prev_ps = [None] * NH
            for grp in range(NG):
                rows = slice(grp * CPG * CH, (grp + 1) * CPG * CH)
                xt = xpool.tile([CH, CPG, D], f32)
                nc.sync.dma_start(
                    out=xt[:],
                    in_=x_d[rows, :].rearrange("(c p) d -> p c d", p=CH),
                )
                ot = opool.tile([CH, CPG, D], f32)
                for c in range(CPG):
                    k = grp * CPG + c
                    for h in range(NH):
                        rhs = xt[:, c, h * DH:(h + 1) * DH]
                        ps = pspool.tile([CH, DH], f32, tag=f"ps{h}")
                        if k > 0:
                            # x_k[0, :] += decay * cache_{k-1}[127, :]
                            # (carry row sits at PSUM partition 0)
                            nc.vector.scalar_tensor_tensor(
                                out=xt[0:1, c, h * DH:(h + 1) * DH],
                                in0=prev_ps[h][0:1, :],
                                scalar=float(decay),
                                in1=xt[0:1, c, h * DH:(h + 1) * DH],
                                op0=mybir.AluOpType.mult,
                                op1=mybir.AluOpType.add,
                            )
                        nc.tensor.matmul(ps[:], lt[:], rhs, start=True, stop=True)
                        nc.scalar.activation(
                            ot[:, c, h * DH:(h + 1) * DH],
                            ps[:],
                            mybir.ActivationFunctionType.Identity,
                            bias=bsb[:, k:k + 1],
                            scale=wsb[:, k:k + 1],
                        )
                        prev_ps[h] = ps
                y_grp = y_d[rows, :].rearrange("(c p) d -> p c d", p=CH)
                nc.sync.dma_start(out=y_grp[0:CH - 1], in_=ot[1:CH])
                nc.sync.dma_start(out=y_grp[CH - 1:CH], in_=ot[0:1])
    nc.compile()
    return nc


def _host_constants(weight, bias, decay):
    """L^T with M rotated by one, plus rotated per-chunk w/b columns."""
    t = np.arange(CH)
    diff = t[:, None] - t[None, :]
    L = np.where(diff >= 0, np.float32(decay) ** diff.astype(np.float32), 0.0)
    L = L.astype(np.float32)
    Lrot = np.roll(L, 1, axis=0)        # row m of Lrot = L row (m-1)%128
    LT = np.ascontiguousarray(Lrot.T)   # lhsT[s, m] = L[(m-1)%128, s]
    WT = np.roll(weight.reshape(NK, CH).T.astype(np.float32), 1, axis=0)
    BT = np.roll(bias.reshape(NK, CH).T.astype(np.float32), 1, axis=0)
    return LT, np.ascontiguousarray(WT), np.ascontiguousarray(BT)


def kernel(x, weight, bias, decay_value):
    from concourse.bass_utils import run_bass_kernel_spmd

    x = np.ascontiguousarray(np.asarray(x, dtype=np.float32))
    weight = np.asarray(weight, dtype=np.float32)
    bias = np.asarray(bias, dtype=np.float32)
    decay = float(np.float32(np.clip(np.asarray(decay_value)[0], 0.9, 1.0)))

    LT, WT, BT = _host_constants(weight, bias, decay)

    key = round(decay, 10)
    if key not in _CACHED:
        _CACHED[key] = _build_program(decay)
    nc = _CACHED[key]

    in_maps = [
        {"x": x[b], "lt": LT, "w": WT, "b": BT} for b in range(B)
    ]
    res = run_bass_kernel_spmd(nc, in_maps, core_ids=list(range(B)))
    out = np.stack([res.results[b]["y"] for b in range(B)], axis=0)
    return out.astype(np.float32)


# revision 4
# speedup vs baseline: 5.0112x; 5.0112x over previous
"""Trainium2 Bass kernel for ColRepeatCausalLinear:

    decay   = clip(decay_value, 0.9, 1.0)
    cache_t = decay * cache_{t-1} + x_t          (scan along T, per (b, d))
    out_t   = weight[t] * cache_t + bias[t]

Shapes: x [B=8, T=4096, D=1024] f32, weight [1, T], bias [T], decay_value [1].

Strategy (one batch per NeuronCore, 8 cores):
  - Chunk T into 32 blocks of 128.  Within a chunk the scan is a matmul
    with the lower-triangular decay matrix L[t, s] = decay^(t-s) (t >= s):
    cache_k = L @ x_k, computed on the TensorEngine in fp32 with the
    chunk rows on partitions and D on the free axis (no transposes).
  - The cross-chunk carry folds into the next chunk's matmul through
    row 0: since L[t, 0] = decay^t, adding decay*carry to x_{k+1}[0, :]
    makes L @ x' produce the full prefix (carry term decay^{t+1}*carry).
    carry = cache_k[127, :].  Engine APs must start on a 32-aligned
    partition, so the matmul's M ordering is rotated by one (PSUM
    partition 0 holds cache[127], partition m holds cache[m-1]); the
    carry patch then reads PSUM partition 0 and the output DMA
    un-rotates (rows 1..127 in one transfer, row 0 in a small second).
  - D is split into two halves (one PSUM bank each) forming two
    independent carry chains, interleaved so the PE never idles on the
    serial patch latency.
  - Final out = weight[t]*cache + bias[t] is a single ScalarEngine
    activation (Identity) with per-partition scale/bias APs, PSUM -> SBUF.
"""

import numpy as np

B, T, D = 8, 4096, 1024
CH = 128                 # chunk rows (PE contraction dim)
NK = T // CH             # 32 chunks
CPG = 4                  # chunks per DMA staging group
NG = NK // CPG           # 8 staging groups
NH = 2                   # d-halves (carry chains)
DH = D // NH             # 512 = one PSUM bank of fp32

_CACHED = {}


def _build_program(decay: float):
    import concourse.bass as bass
    import concourse.mybir as mybir
    from concourse import bacc
    from concourse.tile import TileContext

    f32 = mybir.dt.float32
    nc = bacc.Bacc("TRN2", target_bir_lowering=False)

    x_d = nc.dram_tensor("x", [T, D], f32, kind="ExternalInput")
    lt_d = nc.dram_tensor("lt", [CH, CH], f32, kind="ExternalInput")
    w_d = nc.dram_tensor("w", [CH, NK], f32, kind="ExternalInput")
    b_d = nc.dram_tensor("b", [CH, NK], f32, kind="ExternalInput")
    y_d = nc.dram_tensor("y", [T, D], f32, kind="ExternalOutput")

    with TileContext(nc) as tc:
        with (
            tc.tile_pool(name="const", bufs=1) as const,
            tc.tile_pool(name="xin", bufs=3) as xpool,
            tc.tile_pool(name="oout", bufs=3) as opool,
            tc.tile_pool(name="psum", bufs=2, space="PSUM") as pspool,
        ):
            lt = const.tile([CH, CH], f32)
            nc.sync.dma_start(out=lt[:], in_=lt_d[:])
            wsb = const.tile([CH, NK], f32)
            nc.sync.dma_start(out=wsb[:], in_=w_d[:])
            bsb = const.tile([CH, NK], f32)
            nc.sync.dma_start(out=bsb[:], in_=b_d[:])

            # Output staging layout (per group of CPG chunks): ot has CPG+1
            # slots.  ACT for chunk (CPG*g + c) writes slot c+1 in rotated
            # row order (partition 0 = final row of the chunk, partition p =
            # row p-1).  Slot 0's partition 0 holds the final row of the
            # previous group's last chunk (written by a tiny extra ACT).
            # Then ONE affine 128-partition DMA covers output rows
            # [512g-1, 512g+510] (sprays across all 16 SDMA engines), and a
            # tiny 4-row fixup DMA rewrites the rows the affine map
            # misplaces (the partition-0 lanes).
            prev_ps = [None] * NH
            for grp in range(NG):
                rows = slice(grp * CPG * CH, (grp + 1) * CPG * CH)
                xt = xpool.tile([CH, CPG, D], f32)
                nc.sync.dma_start(
                    out=xt[:],
                    in_=x_d[rows, :].rearrange("(c p) d -> p c d", p=CH),
                )
                ot = opool.tile([CH, CPG + 1, D], f32)
                if grp > 0:
                    kprev = grp * CPG - 1
                    for h in range(NH):
                        nc.scalar.activation(
                            ot[0:1, 0, h * DH:(h + 1) * DH],
                            prev_ps[h][0:1, :],
                            mybir.ActivationFunctionType.Identity,
                            bias=bsb[0:1, kprev:kprev + 1],
                            scale=wsb[0:1, kprev:kprev + 1],
                        )
                for c in range(CPG):
                    k = grp * CPG + c
                    for h in range(NH):
                        rhs = xt[:, c, h * DH:(h + 1) * DH]
                        ps = pspool.tile([CH, DH], f32, tag=f"ps{h}")
                        if k > 0:
                            # x_k[0, :] += decay * cache_{k-1}[127, :]
                            # (carry row sits at PSUM partition 0)
                            nc.vector.scalar_tensor_tensor(
                                out=xt[0:1, c, h * DH:(h + 1) * DH],
                                in0=prev_ps[h][0:1, :],
                                scalar=float(decay),
                                in1=xt[0:1, c, h * DH:(h + 1) * DH],
                                op0=mybir.AluOpType.mult,
                                op1=mybir.AluOpType.add,
                            )
                        nc.tensor.matmul(ps[:], lt[:], rhs, start=True, stop=True)
                        nc.scalar.activation(
                            ot[:, c + 1, h * DH:(h + 1) * DH],
                            ps[:],
                            mybir.ActivationFunctionType.Identity,
                            bias=bsb[:, k:k + 1],
                            scale=wsb[:, k:k + 1],
                        )
                        prev_ps[h] = ps
                r0 = grp * CPG * CH          # first output row of this group
                if grp > 0:
                    # rows [r0-1 .. r0+510]: row = r0 - 1 + 128c + p
                    y_win = y_d[r0 - 1:r0 + CPG * CH - 1, :].rearrange(
                        "(c p) d -> p c d", p=CH)
                    nc.sync.dma_start(out=y_win, in_=ot[:, 1:CPG + 1])
                    # fixup: correct carry rows {r0-1+128c}
                    nc.sync.dma_start(out=y_win[0:1], in_=ot[0:1, 0:CPG])
                else:
                    # group 0: no row -1; chunks 1..3 via the affine window,
                    # chunk 0's body rows 0..126 separately.
                    y_win = y_d[CH - 1:CPG * CH - 1, :].rearrange(
                        "(c p) d -> p c d", p=CH)
                    nc.sync.dma_start(out=y_win, in_=ot[:, 2:CPG + 1])
                    nc.sync.dma_start(out=y_d[0:CH - 1, :], in_=ot[1:CH, 1])
                    # fixup: carry rows {127, 255, 383}
                    nc.sync.dma_start(out=y_win[0:1], in_=ot[0:1, 1:CPG])
            # final output row T-1 = carry row of the last chunk
            nc.sync.dma_start(out=y_d[T - 1:T, :], in_=ot[0:1, CPG, :])
    nc.compile()
    return nc


def _host_constants(weight, bias, decay):
    """L^T with M rotated by one, plus rotated per-chunk w/b columns."""
    t = np.arange(CH)
    diff = t[:, None] - t[None, :]
    L = np.where(diff >= 0, np.float32(decay) ** diff.astype(np.float32), 0.0)
    L = L.astype(np.float32)
    Lrot = np.roll(L, 1, axis=0)        # row m of Lrot = L row (m-1)%128
    LT = np.ascontiguousarray(Lrot.T)   # lhsT[s, m] = L[(m-1)%128, s]
    WT = np.roll(weight.reshape(NK, CH).T.astype(np.float32), 1, axis=0)
    BT = np.roll(bias.reshape(NK, CH).T.astype(np.float32), 1, axis=0)
    return LT, np.ascontiguousarray(WT), np.ascontiguousarray(BT)


def kernel(x, weight, bias, decay_value):
    from concourse.bass_utils import run_bass_kernel_spmd

    x = np.ascontiguousarray(np.asarray(x, dtype=np.float32))
    weight = np.asarray(weight, dtype=np.float32)
    bias = np.asarray(bias, dtype=np.float32)
    decay = float(np.float32(np.clip(np.asarray(decay_value)[0], 0.9, 1.0)))

    LT, WT, BT = _host_constants(weight, bias, decay)

    key = round(decay, 10)
    if key not in _CACHED:
        _CACHED[key] = _build_program(decay)
    nc = _CACHED[key]

    in_maps = [
        {"x": x[b], "lt": LT, "w": WT, "b": BT} for b in range(B)
    ]
    res = run_bass_kernel_spmd(nc, in_maps, core_ids=list(range(B)))
    out = np.stack([res.results[b]["y"] for b in range(B)], axis=0)
    return out.astype(np.float32)
